# revision 13
# baseline (speedup 1.0000x reference)
"""Self-contained Trainium2 Bass kernel for 12-head attention.

Module: out = softmax((xq Wq^T)(xk Wk^T)^T / sqrt(64)) (xv Wv^T) Wp^T + bp
Shapes: xq/xk/xv [2, 2048, 768]; W* [768, 768]; bp [768].

Sharding (8 cores): core c handles batch b = c//4 and head group g = c%4
(3 of the 12 heads).  Each core computes its heads' attention plus the
partial output projection (contraction over its 192 feature columns of
Wp).  Host unshard: out[b] = sum of the 4 group partials + bias.

Per-core dataflow (all matmul operands bf16, fp32 PSUM accumulation):
  Qt/Kt [64, 2048] transposed layout, head pairs packed into 128
  partitions; V [2048, 64] natural + ones column (denominator trick).
  S^T[k,q] = Kt^T Qt via 64-row PE array tiling (two concurrent tiles);
  Et = exp(S*scale) on ScalarE (|S*scale| <= ~3, no max needed);
  Ot[65,q] = V_aug^T Et row-tiled into two PSUM banks; normalize via
  reciprocal + rank-1 PE broadcast; Y[q,768] = OtN^T WpT per q-chunk.
"""

import os
import sys

import numpy as np

for _p in ("/opt/trn_rl_repo",):
    if _p not in sys.path and os.path.isdir(_p):
        sys.path.insert(0, _p)

import ml_dtypes

DIM = 768
NH = 12
HD = 64
N = 2048
B = 2
SCALE = HD ** -0.5
NCORES = 8
HPG = 3  # heads per group (core)

_BUILT = {}
LAST_RESULT = None


def build_bass(row_tile=True):
    import concourse.bacc as bacc
    import concourse.mybir as mybir
    import concourse.tile as tile

    bf16 = mybir.dt.bfloat16
    f32 = mybir.dt.float32
    AF = mybir.ActivationFunctionType

    nc = bacc.Bacc("TRN2", target_bir_lowering=False, debug=False)
    xqT = nc.declare_dram_parameter("xqT", [DIM, N], bf16, isOutput=False)
    xkT = nc.declare_dram_parameter("xkT", [DIM, N], bf16, isOutput=False)
    xvT = nc.declare_dram_parameter("xvT", [DIM, N], bf16, isOutput=False)
    wqkT = nc.declare_dram_parameter("wqkT", [DIM, 2 * 192], bf16, isOutput=False)
    wvT = nc.declare_dram_parameter("wvT", [DIM, 192], bf16, isOutput=False)
    wpT = nc.declare_dram_parameter("wpT", [192, DIM], bf16, isOutput=False)
    out = nc.declare_dram_parameter("out", [N, DIM], f32, isOutput=True)

    KC = DIM // 128  # 6 contraction chunks for projections
    QB = N // 512    # 4 query blocks
    SEQC = N // 128  # 16 sequence chunks

    with tile.TileContext(nc) as tc:
        from contextlib import ExitStack

        with ExitStack() as ctx:
            pX = ctx.enter_context(tc.tile_pool(name="px", bufs=1))
            pW = ctx.enter_context(tc.tile_pool(name="pw", bufs=1))
            pP = ctx.enter_context(tc.tile_pool(name="pp", bufs=1))
            pEt = ctx.enter_context(tc.tile_pool(name="pet", bufs=6))
            pSm = ctx.enter_context(tc.tile_pool(name="psm", bufs=4))
            pY = ctx.enter_context(tc.tile_pool(name="py", bufs=2))
            psS = ctx.enter_context(tc.tile_pool(name="pss", bufs=4, space="PSUM"))
            psO = ctx.enter_context(tc.tile_pool(name="pso", bufs=4, space="PSUM"))

            # ---------------- DMA inputs ----------------
            xq_t, xk_t, xv_t, wqk_t, wv_t = [], [], [], [], []
            for k in range(KC):
                for lst, src, nm, w in (
                    (xq_t, xqT, "xq", N),
                    (xk_t, xkT, "xk", N),
                    (xv_t, xvT, "xv", N),
                    (wqk_t, wqkT, "wqk", 384),
                    (wv_t, wvT, "wv", 192),
                ):
                    t = pX.tile([128, w], bf16, tag=f"{nm}{k}", name=f"{nm}{k}")
                    nc.sync.dma_start(t[:], src[128 * k : 128 * (k + 1), :])
                    lst.append(t)
            wp_t = []
            for h in range(HPG):
                t = pW.tile([128, DIM], bf16, tag=f"wp{h}", name=f"wp{h}")
                nc.sync.dma_start(t[0:64, :], wpT[64 * h : 64 * (h + 1), :])
                wp_t.append(t)
            ones_t = pW.tile([128, 64], bf16, tag="ones")
            nc.gpsimd.memset(ones_t[:], 1.0)

            # ---------------- projections: Qt/Kt ----------------
            # wqkT cols: [0:192] = WqT (heads q0,q1,q2), [192:384] = WkT.
            tA = pP.tile([128, N], bf16, tag="tA")   # [Qt_h0; Qt_h1]
            tB = pP.tile([128, N], bf16, tag="tB")   # [Kt_h0; Kt_h1]
            g3q = pP.tile([128, N], bf16, tag="g3q")  # Qt_h2 duplicated
            g3k = pP.tile([128, N], bf16, tag="g3k")  # Kt_h2 duplicated
            for qb in range(QB):
                qs = slice(512 * qb, 512 * (qb + 1))
                for dst, wc, xt, M in (
                    (tA, 0, xq_t, 128),
                    (tB, 192, xk_t, 128),
                    (g3q, 128, xq_t, 64),
                    (g3k, 320, xk_t, 64),
                ):
                    ps = psS.tile([128, 512], f32, tag="s")
                    for k in range(KC):
                        nc.tensor.matmul(
                            ps[0:M, :],
                            lhsT=wqk_t[k][:, wc : wc + M],
                            rhs=xt[k][:, qs],
                            start=(k == 0),
                            stop=(k == KC - 1),
                        )
                    nc.vector.tensor_copy(dst[0:M, qs], ps[0:M, :])
                    if M == 64:
                        # duplicate head-2 Qt/Kt into partitions 64-127
                        nc.gpsimd.dma_start(dst[64:128, qs], dst[0:64, qs])

            # ---------------- projection: V (natural) + ones columns ----
            v_t = []
            for sc in range(SEQC):
                ps = psS.tile([128, 512], f32, tag="s")
                for k in range(KC):
                    nc.tensor.matmul(
                        ps[:, 0:192],
                        lhsT=xv_t[k][:, 128 * sc : 128 * (sc + 1)],
                        rhs=wv_t[k][:],
                        start=(k == 0),
                        stop=(k == KC - 1),
                    )
                vt = pP.tile([128, 3 * 65], bf16, tag=f"v{sc}", name=f"v{sc}")
                for h in range(HPG):
                    nc.vector.tensor_copy(
                        vt[:, 65 * h : 65 * h + 64], ps[:, 64 * h : 64 * (h + 1)]
                    )
                    nc.gpsimd.memset(vt[:, 65 * h + 64 : 65 * h + 65], 1.0)
                v_t.append(vt)

            # ---------------- attention ----------------
            otn = []
            for h in range(HPG):
                otn.append(pP.tile([128, N], bf16, tag=f"otn{h}", name=f"otn{h}"))

            def tp(pos):
                return pos if row_tile else None

            def s_mm(ps, lhs_t, lhs_sl, rhs_t, rhs_sl, kc, qs, pos):
                lo = 64 * (pos // 64)
                nc.tensor.matmul(
                    ps[:],
                    lhsT=lhs_t[lo : lo + 64, 128 * kc : 128 * (kc + 1)],
                    rhs=rhs_t[lo : lo + 64, qs],
                    start=True,
                    stop=True,
                    tile_position=tp((pos, 0)),
                )

            def pv_mm(po, vt, h, et, kc_first, kc_last, pos):
                lo = pos
                nc.tensor.matmul(
                    po[0:65, :],
                    lhsT=vt[lo : lo + 64, 65 * h : 65 * (h + 1)],
                    rhs=et[lo : lo + 64, :],
                    start=kc_first,
                    stop=kc_last,
                    tile_position=tp((pos, 0)),
                )

            def normalize(h, qb, o_a, o_b):
                # Ot = o_a + o_b (two PSUM banks). DVE may read only one
                # PSUM operand, so ScalarE evacuates o_b to SBUF first.
                qs = slice(512 * qb, 512 * (qb + 1))
                ob_sb = pSm.tile([65, 512], f32, tag="obsb")
                nc.scalar.copy(ob_sb[:], o_b[0:65, :])
                tmp = pSm.tile([65, 512], f32, tag="tmp")
                nc.vector.tensor_add(tmp[:], o_a[0:65, :], ob_sb[:])
                r_sb = pSm.tile([1, 512], bf16, tag="r")
                with nc.allow_low_precision(reason="rank-1 softmax denom bcast"):
                    nc.vector.reciprocal(r_sb[:], tmp[64:65, :])
                rb = psS.tile([128, 512], f32, tag="s")
                nc.tensor.matmul(
                    rb[0:64, :],
                    lhsT=ones_t[0:1, :],
                    rhs=r_sb[0:1, :],
                    start=True,
                    stop=True,
                    tile_position=tp((0, 0)),
                )
                nc.vector.tensor_mul(otn[h][0:64, qs], tmp[0:64, :], rb[0:64, :])

            if row_tile:
                # head pair (h0, h1): two concurrent 64-row PE tiles
                for qb in range(QB):
                    qs = slice(512 * qb, 512 * (qb + 1))
                    o_a = {h: psO.tile([128, 512], f32, tag="o", name=f"oa{h}") for h in (0, 1)}
                    o_b = {h: psO.tile([128, 512], f32, tag="o", name=f"ob{h}") for h in (0, 1)}
                    for kc in range(SEQC):
                        sa = psS.tile([128, 512], f32, tag="s")
                        sb = psS.tile([128, 512], f32, tag="s")
                        s_mm(sa, tB, 0, tA, 0, kc, qs, 0)
                        s_mm(sb, tB, 0, tA, 0, kc, qs, 64)
                        ea = pEt.tile([128, 512], bf16, tag="et")
                        eb = pEt.tile([128, 512], bf16, tag="et")
                        nc.scalar.activation(ea[:], sa[:], AF.Exp, scale=SCALE)
                        nc.scalar.activation(eb[:], sb[:], AF.Exp, scale=SCALE)
                        for h, et in ((0, ea), (1, eb)):
                            pv_mm(o_a[h], v_t[kc], h, et, kc == 0, kc == SEQC - 1, 0)
                            pv_mm(o_b[h], v_t[kc], h, et, kc == 0, kc == SEQC - 1, 64)
                    for h in (0, 1):
                        normalize(h, qb, o_a[h], o_b[h])
                # head 2: pair up k-chunks across the two PE row tiles
                for qb in range(QB):
                    qs = slice(512 * qb, 512 * (qb + 1))
                    o_a = psO.tile([128, 512], f32, tag="o")
                    o_b = psO.tile([128, 512], f32, tag="o")
                    for kp in range(SEQC // 2):
                        k0, k1 = 2 * kp, 2 * kp + 1
                        sa = psS.tile([128, 512], f32, tag="s")
                        sb = psS.tile([128, 512], f32, tag="s")
                        s_mm(sa, g3k, 0, g3q, 0, k0, qs, 0)
                        s_mm(sb, g3k, 0, g3q, 0, k1, qs, 64)
                        ea = pEt.tile([128, 512], bf16, tag="et")
                        eb = pEt.tile([128, 512], bf16, tag="et")
                        nc.scalar.activation(ea[:], sa[:], AF.Exp, scale=SCALE)
                        nc.scalar.activation(eb[:], sb[:], AF.Exp, scale=SCALE)
                        for et, kc in ((ea, k0), (eb, k1)):
                            pv_mm(o_a, v_t[kc], 2, et, kc == 0, kc == SEQC - 1, 0)
                            pv_mm(o_b, v_t[kc], 2, et, kc == 0, kc == SEQC - 1, 64)
                    normalize(2, qb, o_a, o_b)
            else:
                # simple fallback: full 128x128 mode, single PSUM per head
                for h in range(HPG):
                    kt = (tB, g3k)[h // 2]
                    qt = (tA, g3q)[h // 2]
                    lo = 64 * (h % 2)
                    for qb in range(QB):
                        qs = slice(512 * qb, 512 * (qb + 1))
                        o_a = psO.tile([128, 512], f32, tag="o")
                        for kc in range(SEQC):
                            sa = psS.tile([128, 512], f32, tag="s")
                            nc.tensor.matmul(
                                sa[:],
                                lhsT=kt[lo : lo + 64, 128 * kc : 128 * (kc + 1)],
                                rhs=qt[lo : lo + 64, qs],
                                start=True,
                                stop=True,
                            )
                            ea = pEt.tile([128, 512], bf16, tag="et")
                            nc.scalar.activation(ea[:], sa[:], AF.Exp, scale=SCALE)
                            nc.tensor.matmul(
                                o_a[0:65, :],
                                lhsT=v_t[kc][:, 65 * h : 65 * (h + 1)],
                                rhs=ea[:],
                                start=(kc == 0),
                                stop=(kc == SEQC - 1),
                            )
                        tmp = pSm.tile([65, 512], f32, tag="tmp")
                        nc.scalar.copy(tmp[:], o_a[0:65, :])
                        r_sb = pSm.tile([1, 512], bf16, tag="r")
                        with nc.allow_low_precision(reason="softmax denom"):
                            nc.vector.reciprocal(r_sb[:], tmp[64:65, :])
                        rb = psS.tile([128, 512], f32, tag="s")
                        nc.tensor.matmul(
                            rb[0:64, :],
                            lhsT=ones_t[0:1, :],
                            rhs=r_sb[0:1, :],
                            start=True,
                            stop=True,
                        )
                        nc.vector.tensor_mul(
                            otn[h][0:64, slice(512 * qb, 512 * (qb + 1))],
                            tmp[0:64, :],
                            rb[0:64, :],
                        )

            # ---------------- output projection ----------------
            for qc in range(SEQC):
                cs = slice(128 * qc, 128 * (qc + 1))
                y1 = psS.tile([128, 512], f32, tag="s")
                y2 = psS.tile([128, 512], f32, tag="s")
                for ps, off, w in ((y1, 0, 512), (y2, 512, 256)):
                    for h in range(HPG):
                        nc.tensor.matmul(
                            ps[:, 0:w],
                            lhsT=otn[h][0:64, cs],
                            rhs=wp_t[h][0:64, off : off + w],
                            start=(h == 0),
                            stop=(h == HPG - 1),
                        )
                y_sb = pY.tile([128, DIM], f32, tag="y")
                nc.vector.tensor_copy(y_sb[:, 0:512], y1[:, 0:512])
                nc.vector.tensor_copy(y_sb[:, 512:768], y2[:, 0:256])
                nc.sync.dma_start(out[cs, :], y_sb[:])

    nc.compile()
    return nc


def _shard_inputs(xq, xk, xv, Wq, Wk, Wv, Wp):
    bf = ml_dtypes.bfloat16
    in_maps = []
    xT = {}
    for b in range(B):
        xT[b] = tuple(
            np.ascontiguousarray(a[b].T).astype(bf) for a in (xq, xk, xv)
        )
    for c in range(NCORES):
        b, g = c // 4, c % 4
        hs = slice(192 * g, 192 * (g + 1))
        wqk = np.concatenate([Wq[hs].T, Wk[hs].T], axis=1)
        in_maps.append(
            {
                "xqT": xT[b][0],
                "xkT": xT[b][1],
                "xvT": xT[b][2],
                "wqkT": np.ascontiguousarray(wqk).astype(bf),
                "wvT": np.ascontiguousarray(Wv[hs].T).astype(bf),
                "wpT": np.ascontiguousarray(Wp[:, hs].T).astype(bf),
            }
        )
    return in_maps


def _ensure_ntff_hook():
    """Register the axon NTFF profiling hook if the stub antenv lacks it."""
    import types

    try:
        from antenv.axon_hooks import get_axon_ntff_profile_hook  # noqa: F401

        return
    except ImportError:
        pass
    try:
        import antenv
        from trn_agent_boot.trn_boot import _ntff_profile_via_ctypes

        so_path = "/opt/axon/libaxon_pjrt.so"
        hook = _ntff_profile_via_ctypes(so_path) if os.path.exists(so_path) else None
        mod = types.ModuleType("antenv.axon_hooks")
        _state = {"h": hook}
        mod.get_axon_ntff_profile_hook = lambda: _state["h"]
        mod.set_axon_ntff_profile_hook = lambda h: _state.__setitem__("h", h)
        sys.modules["antenv.axon_hooks"] = mod
        antenv.axon_hooks = mod
    except Exception:
        pass


def kernel(xq, xk, xv, Wq, Wk, Wv, Wp, bp):
    global LAST_RESULT
    from concourse.bass_utils import run_bass_kernel_spmd

    key = "nc"
    if key not in _BUILT:
        _BUILT[key] = build_bass()
    nc = _BUILT[key]

    xq, xk, xv = (np.asarray(a, np.float32) for a in (xq, xk, xv))
    Wq, Wk, Wv, Wp = (np.asarray(a, np.float32) for a in (Wq, Wk, Wv, Wp))
    bp = np.asarray(bp, np.float32)

    in_maps = _shard_inputs(xq, xk, xv, Wq, Wk, Wv, Wp)
    trace = bool(os.environ.get("BASS_KERNEL_TRACE"))
    if trace:
        _ensure_ntff_hook()
    res = run_bass_kernel_spmd(
        nc, in_maps, core_ids=list(range(NCORES)), trace=trace
    )
    LAST_RESULT = res
    parts = [res.results[i]["out"].astype(np.float32) for i in range(NCORES)]
    out = np.stack(
        [
            parts[0] + parts[1] + parts[2] + parts[3],
            parts[4] + parts[5] + parts[6] + parts[7],
        ]
    )
    return (out + bp[None, None, :]).astype(np.float32)


# revision 15
# speedup vs baseline: 1.0707x; 1.0707x over previous
"""Self-contained Trainium2 Bass kernel for 12-head attention.

Module: out = softmax((xq Wq^T)(xk Wk^T)^T / sqrt(64)) (xv Wv^T) Wp^T + bp
Shapes: xq/xk/xv [2, 2048, 768]; W* [768, 768]; bp [768].

Sharding (8 cores): core c handles batch b = c//4 and head group g = c%4
(3 of the 12 heads).  Each core computes its heads' attention plus the
partial output projection (contraction over its 192 feature columns of
Wp).  Host unshard: out[b] = sum of the 4 group partials + bias.

Per-core dataflow (all matmul operands bf16, fp32 PSUM accumulation):
  Qt/Kt [64, 2048] transposed layout, head pairs packed into 128
  partitions; V [2048, 64] natural + ones column (denominator trick).
  S^T[k,q] = Kt^T Qt via 64-row PE array tiling (two concurrent tiles);
  Et = exp(S*scale) on ScalarE (|S*scale| <= ~3, no max needed);
  Ot[65,q] = V_aug^T Et row-tiled into two PSUM banks; normalize via
  reciprocal + rank-1 PE broadcast; Y[q,768] = OtN^T WpT per q-chunk.
"""

import os
import sys

import numpy as np

for _p in ("/opt/trn_rl_repo",):
    if _p not in sys.path and os.path.isdir(_p):
        sys.path.insert(0, _p)

import ml_dtypes

DIM = 768
NH = 12
HD = 64
N = 2048
B = 2
SCALE = HD ** -0.5
NCORES = 8
HPG = 3  # heads per group (core)

_BUILT = {}
LAST_RESULT = None


def build_bass():
    import concourse.bacc as bacc
    import concourse.mybir as mybir
    import concourse.tile as tile

    bf16 = mybir.dt.bfloat16
    f32 = mybir.dt.float32
    AF = mybir.ActivationFunctionType

    nc = bacc.Bacc("TRN2", target_bir_lowering=False, debug=False)
    xqT = nc.declare_dram_parameter("xqT", [DIM, N], bf16, isOutput=False)
    xkT = nc.declare_dram_parameter("xkT", [DIM, N], bf16, isOutput=False)
    xvT = nc.declare_dram_parameter("xvT", [DIM, N], bf16, isOutput=False)
    wqkT = nc.declare_dram_parameter("wqkT", [DIM, 2 * 192], bf16, isOutput=False)
    wvT = nc.declare_dram_parameter("wvT", [DIM, 192], bf16, isOutput=False)
    wpT = nc.declare_dram_parameter("wpT", [192, DIM], bf16, isOutput=False)
    out = nc.declare_dram_parameter("out", [N, DIM], f32, isOutput=True)

    KC = DIM // 128  # 6 contraction chunks for projections
    QB = N // 512    # 4 query blocks
    SEQC = N // 128  # 16 sequence chunks

    with tile.TileContext(nc) as tc:
        from contextlib import ExitStack

        with ExitStack() as ctx:
            pX = ctx.enter_context(tc.tile_pool(name="px", bufs=1))
            pW = ctx.enter_context(tc.tile_pool(name="pw", bufs=1))
            pP = ctx.enter_context(tc.tile_pool(name="pp", bufs=1))
            pEt = ctx.enter_context(tc.tile_pool(name="pet", bufs=6))
            pSm = ctx.enter_context(tc.tile_pool(name="psm", bufs=4))
            pY = ctx.enter_context(tc.tile_pool(name="py", bufs=2))
            psS2 = ctx.enter_context(tc.tile_pool(name="pss2", bufs=2, space="PSUM"))
            psO = ctx.enter_context(tc.tile_pool(name="pso", bufs=4, space="PSUM"))

            # ---------------- DMA inputs ----------------
            xq_t, xk_t, xv_t, wqk_t, wv_t = [], [], [], [], []
            for k in range(KC):
                for lst, src, nm, w in (
                    (xq_t, xqT, "xq", N),
                    (xk_t, xkT, "xk", N),
                    (xv_t, xvT, "xv", N),
                    (wqk_t, wqkT, "wqk", 384),
                    (wv_t, wvT, "wv", 192),
                ):
                    t = pX.tile([128, w], bf16, tag=f"{nm}{k}", name=f"{nm}{k}")
                    nc.sync.dma_start(t[:], src[128 * k : 128 * (k + 1), :])
                    lst.append(t)
            wp_t = []
            for h in range(HPG):
                t = pW.tile([128, DIM], bf16, tag=f"wp{h}", name=f"wp{h}")
                nc.sync.dma_start(t[0:64, :], wpT[64 * h : 64 * (h + 1), :])
                wp_t.append(t)
            ones_t = pW.tile([128, 64], bf16, tag="ones")
            nc.gpsimd.memset(ones_t[:], 1.0)

            # ---------------- projections: Qt/Kt ----------------
            # wqkT cols: [0:192] = WqT (heads q0,q1,q2), [192:384] = WkT.
            tA = pP.tile([128, N], bf16, tag="tA")   # [Qt_h0; Qt_h1]
            tB = pP.tile([128, N], bf16, tag="tB")   # [Kt_h0; Kt_h1]
            g3q = pP.tile([128, N], bf16, tag="g3q")  # Qt_h2 duplicated
            g3k = pP.tile([128, N], bf16, tag="g3k")  # Kt_h2 duplicated
            for qb in range(QB):
                qs = slice(512 * qb, 512 * (qb + 1))
                for dst, wc, xt, M in (
                    (tA, 0, xq_t, 128),
                    (tB, 192, xk_t, 128),
                    (g3q, 128, xq_t, 64),
                    (g3k, 320, xk_t, 64),
                ):
                    ps = psO.tile([128, 512], f32, tag="o", name="ps_qk")
                    for k in range(KC):
                        nc.tensor.matmul(
                            ps[0:M, :],
                            lhsT=wqk_t[k][:, wc : wc + M],
                            rhs=xt[k][:, qs],
                            start=(k == 0),
                            stop=(k == KC - 1),
                        )
                    nc.vector.tensor_copy(dst[0:M, qs], ps[0:M, :])
                    if M == 64:
                        # duplicate head-2 Qt/Kt into partitions 64-127
                        nc.gpsimd.dma_start(dst[64:128, qs], dst[0:64, qs])

            # ---------------- projection: V (natural) + ones columns ----
            v_t = []
            for sc in range(SEQC):
                ps = psO.tile([128, 512], f32, tag="o", name="ps_v")
                for k in range(KC):
                    nc.tensor.matmul(
                        ps[:, 0:192],
                        lhsT=xv_t[k][:, 128 * sc : 128 * (sc + 1)],
                        rhs=wv_t[k][:],
                        start=(k == 0),
                        stop=(k == KC - 1),
                    )
                vt = pP.tile([128, 3 * 65], bf16, tag=f"v{sc}", name=f"v{sc}")
                for h in range(HPG):
                    nc.vector.tensor_copy(
                        vt[:, 65 * h : 65 * h + 64], ps[:, 64 * h : 64 * (h + 1)]
                    )
                    nc.gpsimd.memset(vt[:, 65 * h + 64 : 65 * h + 65], 1.0)
                v_t.append(vt)

            # ---------------- attention ----------------
            otn = []
            for h in range(HPG):
                otn.append(pP.tile([128, N], bf16, tag=f"otn{h}", name=f"otn{h}"))

            def normalize(h, qb, o_a, o_b):
                # Ot = o_a + o_b (two PSUM banks). DVE may read only one
                # PSUM operand, so ScalarE evacuates o_b to SBUF first.
                qs = slice(512 * qb, 512 * (qb + 1))
                ob_sb = pSm.tile([65, 512], f32, tag="obsb")
                nc.scalar.copy(ob_sb[:], o_b[0:65, :])
                tmp = pSm.tile([65, 512], f32, tag="tmp")
                nc.vector.tensor_add(tmp[:], o_a[0:65, :], ob_sb[:])
                r_sb = pSm.tile([1, 512], bf16, tag="r")
                with nc.allow_low_precision(reason="softmax denom bcast"):
                    nc.vector.reciprocal(r_sb[:], tmp[64:65, :])
                rb = psO.tile([128, 512], f32, tag="o", name="rb")
                nc.tensor.matmul(
                    rb[0:64, :],
                    lhsT=ones_t[0:1, :],
                    rhs=r_sb[0:1, :],
                    start=True,
                    stop=True,
                    tile_position=(0, 0),
                )
                nc.vector.tensor_mul(otn[h][0:64, qs], tmp[0:64, :], rb[0:64, :])

            # Units: u=0 -> heads (0,1) packed in tA/tB halves; u=1 -> head 2
            # with even/odd k-chunks on the two PE row tiles (g3q/g3k dup'd).
            for u in range(2):
                for qb in range(QB):
                    qs = slice(512 * qb, 512 * (qb + 1))
                    o_a = {}
                    o_b = {}
                    for i in range(2 if u == 0 else 1):
                        o_a[i] = psO.tile([128, 512], f32, tag="o", name=f"oa{i}")
                        o_b[i] = psO.tile([128, 512], f32, tag="o", name=f"ob{i}")

                    def emit_pv(et, step):
                        # column half c of et -> (head index, k-chunk)
                        for c in (0, 1):
                            cs = slice(512 * c, 512 * (c + 1))
                            if u == 0:
                                hh, kc, oi = c, step, c
                            else:
                                hh, kc, oi = 2, 2 * step + c, 0
                            vsl = slice(65 * hh, 65 * hh + 65)
                            nc.tensor.matmul(
                                o_a[oi][0:65, :],
                                lhsT=v_t[kc][0:64, vsl],
                                rhs=et[0:64, cs],
                                start=(kc == 0),
                                stop=(kc == SEQC - 1),
                                tile_position=(0, 0),
                            )
                            nc.tensor.matmul(
                                o_b[oi][0:65, :],
                                lhsT=v_t[kc][64:128, vsl],
                                rhs=et[64:128, cs],
                                start=(kc == 0),
                                stop=(kc == SEQC - 1),
                                tile_position=(64, 0),
                            )

                    nsteps = SEQC if u == 0 else SEQC // 2
                    pend = None
                    for step in range(nsteps):
                        s2 = psS2.tile([128, 1024], f32, tag="s2", name="s2")
                        if u == 0:
                            # halves: head0 S(kc) | head1 S(kc)
                            nc.tensor.matmul(
                                s2[:, 0:512],
                                lhsT=tB[0:64, 128 * step : 128 * (step + 1)],
                                rhs=tA[0:64, qs],
                                start=True,
                                stop=True,
                                tile_position=(0, 0),
                            )
                            nc.tensor.matmul(
                                s2[:, 512:1024],
                                lhsT=tB[64:128, 128 * step : 128 * (step + 1)],
                                rhs=tA[64:128, qs],
                                start=True,
                                stop=True,
                                tile_position=(64, 0),
                            )
                        else:
                            # halves: head2 S(2*step) | head2 S(2*step+1)
                            k0, k1 = 2 * step, 2 * step + 1
                            nc.tensor.matmul(
                                s2[:, 0:512],
                                lhsT=g3k[0:64, 128 * k0 : 128 * (k0 + 1)],
                                rhs=g3q[0:64, qs],
                                start=True,
                                stop=True,
                                tile_position=(0, 0),
                            )
                            nc.tensor.matmul(
                                s2[:, 512:1024],
                                lhsT=g3k[64:128, 128 * k1 : 128 * (k1 + 1)],
                                rhs=g3q[64:128, qs],
                                start=True,
                                stop=True,
                                tile_position=(64, 0),
                            )
                        et = pEt.tile([128, 1024], bf16, tag="et", name="et")
                        nc.scalar.activation(et[:], s2[:], AF.Exp, scale=SCALE)
                        if pend is not None:
                            emit_pv(*pend)
                        pend = (et, step)
                    emit_pv(*pend)
                    if u == 0:
                        normalize(0, qb, o_a[0], o_b[0])
                        normalize(1, qb, o_a[1], o_b[1])
                    else:
                        normalize(2, qb, o_a[0], o_b[0])

            # ---------------- output projection ----------------
            for qc in range(SEQC):
                cs = slice(128 * qc, 128 * (qc + 1))
                y1 = psO.tile([128, 512], f32, tag="o", name="y1")
                y2 = psO.tile([128, 512], f32, tag="o", name="y2")
                for ps, off, w in ((y1, 0, 512), (y2, 512, 256)):
                    for h in range(HPG):
                        nc.tensor.matmul(
                            ps[:, 0:w],
                            lhsT=otn[h][0:64, cs],
                            rhs=wp_t[h][0:64, off : off + w],
                            start=(h == 0),
                            stop=(h == HPG - 1),
                        )
                y_sb = pY.tile([128, DIM], f32, tag="y")
                nc.vector.tensor_copy(y_sb[:, 0:512], y1[:, 0:512])
                nc.vector.tensor_copy(y_sb[:, 512:768], y2[:, 0:256])
                nc.sync.dma_start(out[cs, :], y_sb[:])

    nc.compile()
    return nc


def _shard_inputs(xq, xk, xv, Wq, Wk, Wv, Wp):
    bf = ml_dtypes.bfloat16
    in_maps = []
    xT = {}
    for b in range(B):
        xT[b] = tuple(
            np.ascontiguousarray(a[b].T).astype(bf) for a in (xq, xk, xv)
        )
    for c in range(NCORES):
        b, g = c // 4, c % 4
        hs = slice(192 * g, 192 * (g + 1))
        wqk = np.concatenate([Wq[hs].T, Wk[hs].T], axis=1)
        in_maps.append(
            {
                "xqT": xT[b][0],
                "xkT": xT[b][1],
                "xvT": xT[b][2],
                "wqkT": np.ascontiguousarray(wqk).astype(bf),
                "wvT": np.ascontiguousarray(Wv[hs].T).astype(bf),
                "wpT": np.ascontiguousarray(Wp[:, hs].T).astype(bf),
            }
        )
    return in_maps


def _ensure_ntff_hook():
    """Register the axon NTFF profiling hook if the stub antenv lacks it."""
    import types

    try:
        from antenv.axon_hooks import get_axon_ntff_profile_hook  # noqa: F401

        return
    except ImportError:
        pass
    try:
        import antenv
        from trn_agent_boot.trn_boot import _ntff_profile_via_ctypes

        so_path = "/opt/axon/libaxon_pjrt.so"
        hook = _ntff_profile_via_ctypes(so_path) if os.path.exists(so_path) else None
        mod = types.ModuleType("antenv.axon_hooks")
        _state = {"h": hook}
        mod.get_axon_ntff_profile_hook = lambda: _state["h"]
        mod.set_axon_ntff_profile_hook = lambda h: _state.__setitem__("h", h)
        sys.modules["antenv.axon_hooks"] = mod
        antenv.axon_hooks = mod
    except Exception:
        pass


def kernel(xq, xk, xv, Wq, Wk, Wv, Wp, bp):
    global LAST_RESULT
    from concourse.bass_utils import run_bass_kernel_spmd

    key = "nc"
    if key not in _BUILT:
        _BUILT[key] = build_bass()
    nc = _BUILT[key]

    xq, xk, xv = (np.asarray(a, np.float32) for a in (xq, xk, xv))
    Wq, Wk, Wv, Wp = (np.asarray(a, np.float32) for a in (Wq, Wk, Wv, Wp))
    bp = np.asarray(bp, np.float32)

    in_maps = _shard_inputs(xq, xk, xv, Wq, Wk, Wv, Wp)
    trace = bool(os.environ.get("BASS_KERNEL_TRACE"))
    if trace:
        _ensure_ntff_hook()
    res = run_bass_kernel_spmd(
        nc, in_maps, core_ids=list(range(NCORES)), trace=trace
    )
    LAST_RESULT = res
    parts = [res.results[i]["out"].astype(np.float32) for i in range(NCORES)]
    out = np.stack(
        [
            parts[0] + parts[1] + parts[2] + parts[3],
            parts[4] + parts[5] + parts[6] + parts[7],
        ]
    )
    return (out + bp[None, None, :]).astype(np.float32)


# revision 18
# speedup vs baseline: 1.2423x; 1.1603x over previous
"""Self-contained Trainium2 Bass kernel for 12-head attention.

Module: out = softmax((xq Wq^T)(xk Wk^T)^T / sqrt(64)) (xv Wv^T) Wp^T + bp
Shapes: xq/xk/xv [2, 2048, 768]; W* [768, 768]; bp [768].

Sharding (8 cores): core c handles batch b = c//4 and head group g = c%4
(3 of the 12 heads).  Each core computes its heads' attention plus the
partial output projection (contraction over its 192 feature columns of
Wp).  Host unshard: out[b] = sum of the 4 group partials + bias.

Per-core dataflow (all matmul operands bf16, fp32 PSUM accumulation):
  Qt/Kt [64, 2048] transposed layout, head pairs packed into 128
  partitions; V [2048, 64] natural + ones column (denominator trick).
  S^T[k,q] = Kt^T Qt via 64-row PE array tiling (two concurrent tiles);
  Et = exp(S*scale) on ScalarE (|S*scale| <= ~3, no max needed);
  Ot[65,q] = V_aug^T Et row-tiled into two PSUM banks; normalize via
  reciprocal + rank-1 PE broadcast; Y[q,768] = OtN^T WpT per q-chunk.
"""

import os
import sys

import numpy as np

for _p in ("/opt/trn_rl_repo",):
    if _p not in sys.path and os.path.isdir(_p):
        sys.path.insert(0, _p)

import ml_dtypes

DIM = 768
NH = 12
HD = 64
N = 2048
B = 2
SCALE = HD ** -0.5
NCORES = 8
HPG = 3  # heads per group (core)

_BUILT = {}
LAST_RESULT = None


def build_bass():
    import concourse.bacc as bacc
    import concourse.mybir as mybir
    import concourse.tile as tile

    bf16 = mybir.dt.bfloat16
    f32 = mybir.dt.float32
    AF = mybir.ActivationFunctionType

    nc = bacc.Bacc("TRN2", target_bir_lowering=False, debug=False)
    xqT = nc.declare_dram_parameter("xqT", [DIM, N], bf16, isOutput=False)
    xkT = nc.declare_dram_parameter("xkT", [DIM, N], bf16, isOutput=False)
    xvT = nc.declare_dram_parameter("xvT", [DIM, N], bf16, isOutput=False)
    wqkT = nc.declare_dram_parameter("wqkT", [DIM, 2 * 192], bf16, isOutput=False)
    wvT = nc.declare_dram_parameter("wvT", [DIM, 192], bf16, isOutput=False)
    wpT = nc.declare_dram_parameter("wpT", [192, DIM], bf16, isOutput=False)
    out = nc.declare_dram_parameter("out", [N, DIM], f32, isOutput=True)

    KC = DIM // 128  # 6 contraction chunks for projections
    QB = N // 512    # 4 query blocks
    SEQC = N // 128  # 16 sequence chunks

    with tile.TileContext(nc) as tc:
        from contextlib import ExitStack

        with ExitStack() as ctx:
            pX = ctx.enter_context(tc.tile_pool(name="px", bufs=1))
            pW = ctx.enter_context(tc.tile_pool(name="pw", bufs=1))
            pP = ctx.enter_context(tc.tile_pool(name="pp", bufs=1))
            pEt = ctx.enter_context(tc.tile_pool(name="pet", bufs=6))
            pSm = ctx.enter_context(tc.tile_pool(name="psm", bufs=4))
            pY = ctx.enter_context(tc.tile_pool(name="py", bufs=2))
            psS2 = ctx.enter_context(tc.tile_pool(name="pss2", bufs=2, space="PSUM"))
            psO = ctx.enter_context(tc.tile_pool(name="pso", bufs=4, space="PSUM"))

            # ---------------- DMA inputs ----------------
            xq_t, xk_t, xv_t, wqk_t, wv_t = [], [], [], [], []
            for k in range(KC):
                for lst, src, nm, w in (
                    (xq_t, xqT, "xq", N),
                    (xk_t, xkT, "xk", N),
                    (xv_t, xvT, "xv", N),
                    (wqk_t, wqkT, "wqk", 384),
                    (wv_t, wvT, "wv", 192),
                ):
                    t = pX.tile([128, w], bf16, tag=f"{nm}{k}", name=f"{nm}{k}")
                    nc.sync.dma_start(t[:], src[128 * k : 128 * (k + 1), :])
                    lst.append(t)
            wp_t = []
            for h in range(HPG):
                t = pW.tile([128, DIM], bf16, tag=f"wp{h}", name=f"wp{h}")
                nc.sync.dma_start(t[0:64, :], wpT[64 * h : 64 * (h + 1), :])
                wp_t.append(t)
            ones_t = pW.tile([128, 64], bf16, tag="ones")
            nc.gpsimd.memset(ones_t[:], 1.0)

            # ---------------- projections: Qt/Kt ----------------
            # wqkT cols: [0:192] = WqT (heads q0,q1,q2), [192:384] = WkT.
            tA = pP.tile([128, N], bf16, tag="tA")   # [Qt_h0; Qt_h1]
            tB = pP.tile([128, N], bf16, tag="tB")   # [Kt_h0; Kt_h1]
            g3q = pP.tile([128, N], bf16, tag="g3q")  # Qt_h2 duplicated
            g3k = pP.tile([128, N], bf16, tag="g3k")  # Kt_h2 duplicated
            for qb in range(QB):
                qs = slice(512 * qb, 512 * (qb + 1))
                for dst, wc, xt in ((tA, 0, xq_t), (tB, 192, xk_t)):
                    ps = psO.tile([128, 512], f32, tag="o", name="ps_qk")
                    for k in range(KC):
                        nc.tensor.matmul(
                            ps[:],
                            lhsT=wqk_t[k][:, wc : wc + 128],
                            rhs=xt[k][:, qs],
                            start=(k == 0),
                            stop=(k == KC - 1),
                        )
                    nc.vector.tensor_copy(dst[:, qs], ps[:])
                # head-2 Qt/Kt: two concurrent 64-col PE tiles -> one PSUM
                ps = psO.tile([128, 512], f32, tag="o", name="ps_g3")
                for k in range(KC):
                    nc.tensor.matmul(
                        ps[0:64, :],
                        lhsT=wqk_t[k][:, 128:192],
                        rhs=xq_t[k][:, qs],
                        start=(k == 0),
                        stop=(k == KC - 1),
                        tile_position=(0, 0),
                        skip_group_check=True,
                    )
                    nc.tensor.matmul(
                        ps[64:128, :],
                        lhsT=wqk_t[k][:, 320:384],
                        rhs=xk_t[k][:, qs],
                        start=(k == 0),
                        stop=(k == KC - 1),
                        tile_position=(0, 64),
                        skip_group_check=True,
                    )
                nc.vector.tensor_copy(g3q[0:64, qs], ps[0:64, :])
                nc.vector.tensor_copy(g3k[64:128, qs], ps[64:128, :])
                # partition-shifting duplicates (DMA can cross partitions)
                nc.gpsimd.dma_start(g3q[64:128, qs], g3q[0:64, qs])
                nc.gpsimd.dma_start(g3k[0:64, qs], g3k[64:128, qs])

            # ---------------- projection: V (natural) + ones columns ----
            v_t = []
            for sc in range(SEQC):
                ps = psO.tile([128, 512], f32, tag="o", name="ps_v")
                for k in range(KC):
                    nc.tensor.matmul(
                        ps[:, 0:192],
                        lhsT=xv_t[k][:, 128 * sc : 128 * (sc + 1)],
                        rhs=wv_t[k][:],
                        start=(k == 0),
                        stop=(k == KC - 1),
                    )
                vt = pP.tile([128, 3 * 65], bf16, tag=f"v{sc}", name=f"v{sc}")
                for h in range(HPG):
                    nc.vector.tensor_copy(
                        vt[:, 65 * h : 65 * h + 64], ps[:, 64 * h : 64 * (h + 1)]
                    )
                    nc.gpsimd.memset(vt[:, 65 * h + 64 : 65 * h + 65], 1.0)
                v_t.append(vt)

            # ---------------- attention ----------------
            otn = []
            for h in range(HPG):
                otn.append(pP.tile([128, N], bf16, tag=f"otn{h}", name=f"otn{h}"))
            tmagic = []
            for i in range(12):
                tmagic.append(
                    pSm.tile([65, 512], f32, tag=f"tmp{i}", bufs=1, name=f"tmp{i}")
                )
            den_q = []
            for qb in range(QB):
                den_q.append(
                    pSm.tile([3, 512], f32, tag=f"den{qb}", bufs=1, name=f"den{qb}")
                )

            def attn_unit(u, qb):
                """Emit S->exp->PV pipeline for unit u, query block qb.
                Returns the live O psum tiles per head-in-unit."""
                qs = slice(512 * qb, 512 * (qb + 1))
                o_a, o_b = {}, {}
                for i in range(2 if u == 0 else 1):
                    o_a[i] = psO.tile([128, 512], f32, tag="o", name=f"oa{i}")
                    o_b[i] = psO.tile([128, 512], f32, tag="o", name=f"ob{i}")

                def emit_pv(et, step):
                    for c in (0, 1):
                        cs = slice(512 * c, 512 * (c + 1))
                        if u == 0:
                            hh, kc, oi = c, step, c
                        else:
                            hh, kc, oi = 2, 2 * step + c, 0
                        vsl = slice(65 * hh, 65 * hh + 65)
                        nc.tensor.matmul(
                            o_a[oi][0:65, :],
                            lhsT=v_t[kc][0:64, vsl],
                            rhs=et[0:64, cs],
                            start=(kc == 0),
                            stop=(kc == SEQC - 1),
                            tile_position=(0, 0),
                        )
                        nc.tensor.matmul(
                            o_b[oi][0:65, :],
                            lhsT=v_t[kc][64:128, vsl],
                            rhs=et[64:128, cs],
                            start=(kc == 0),
                            stop=(kc == SEQC - 1),
                            tile_position=(64, 0),
                        )

                nsteps = SEQC if u == 0 else SEQC // 2
                pend = None
                for step in range(nsteps):
                    s2 = psS2.tile([128, 1024], f32, tag="s2", name="s2")
                    if u == 0:
                        lt, rt = tB, tA
                        k0 = k1 = step
                    else:
                        lt, rt = g3k, g3q
                        k0, k1 = 2 * step, 2 * step + 1
                    nc.tensor.matmul(
                        s2[:, 0:512],
                        lhsT=lt[0:64, 128 * k0 : 128 * (k0 + 1)],
                        rhs=rt[0:64, qs],
                        start=True,
                        stop=True,
                        tile_position=(0, 0),
                    )
                    nc.tensor.matmul(
                        s2[:, 512:1024],
                        lhsT=lt[64:128, 128 * k1 : 128 * (k1 + 1)],
                        rhs=rt[64:128, qs],
                        start=True,
                        stop=True,
                        tile_position=(64, 0),
                    )
                    et = pEt.tile([128, 1024], bf16, tag="et", name="et")
                    nc.scalar.activation(et[:], s2[:], AF.Exp, scale=SCALE)
                    if pend is not None:
                        emit_pv(*pend)
                    pend = (et, step)
                emit_pv(*pend)
                return o_a, o_b

            def collect(h, qb, o_a, o_b):
                """Phase A: merge the two PV banks, stash denominator row.
                Frees the O psum slots quickly; reciprocal+bcast deferred."""
                idx = 3 * qb + h
                ob_sb = pSm.tile([65, 512], f32, tag="obsb")
                nc.scalar.copy(ob_sb[:], o_b[0:65, :])
                nc.vector.tensor_add(tmagic[idx][:], o_a[0:65, :], ob_sb[:])
                # gpsimd DMA shifts the denom row to partition h of den_q[qb]
                nc.gpsimd.dma_start(
                    den_q[qb][h : h + 1, :], tmagic[idx][64:65, :]
                )

            def finalize(qb):
                """Phase B: batched reciprocal + rank-1 bcast + normalize."""
                qs = slice(512 * qb, 512 * (qb + 1))
                r3 = pSm.tile([3, 512], bf16, tag="r3")
                with nc.allow_low_precision(reason="softmax denom bcast"):
                    nc.vector.reciprocal(r3[:], den_q[qb][:])
                for h in range(HPG):
                    idx = 3 * qb + h
                    # matmul operands must start at partition 0/32/64:
                    # DMA-shift row h of r3 down to partition 0
                    r1 = pSm.tile([1, 512], bf16, tag="r1")
                    nc.gpsimd.dma_start(r1[:], r3[h : h + 1, :])
                    rb = psO.tile([128, 512], f32, tag="o", name="rb")
                    nc.tensor.matmul(
                        rb[0:64, :],
                        lhsT=ones_t[0:1, :],
                        rhs=r1[0:1, :],
                        start=True,
                        stop=True,
                        tile_position=(0, 0),
                    )
                    nc.vector.tensor_mul(
                        otn[h][0:64, qs], tmagic[idx][0:64, :], rb[0:64, :]
                    )

            for qb in range(QB):
                o_a0, o_b0 = attn_unit(0, qb)
                if qb > 0:
                    finalize(qb - 1)
                o_a1, o_b1 = attn_unit(1, qb)
                collect(0, qb, o_a0[0], o_b0[0])
                collect(1, qb, o_a0[1], o_b0[1])
                collect(2, qb, o_a1[0], o_b1[0])
            finalize(QB - 1)

            # ---------------- output projection ----------------
            for qc in range(SEQC):
                cs = slice(128 * qc, 128 * (qc + 1))
                y1 = psO.tile([128, 512], f32, tag="o", name="y1")
                y2 = psO.tile([128, 512], f32, tag="o", name="y2")
                for ps, off, w in ((y1, 0, 512), (y2, 512, 256)):
                    for h in range(HPG):
                        nc.tensor.matmul(
                            ps[:, 0:w],
                            lhsT=otn[h][0:64, cs],
                            rhs=wp_t[h][0:64, off : off + w],
                            start=(h == 0),
                            stop=(h == HPG - 1),
                        )
                y_sb = pY.tile([128, DIM], f32, tag="y")
                nc.vector.tensor_copy(y_sb[:, 0:512], y1[:, 0:512])
                nc.vector.tensor_copy(y_sb[:, 512:768], y2[:, 0:256])
                nc.sync.dma_start(out[cs, :], y_sb[:])

    nc.compile()
    return nc


def _shard_inputs(xq, xk, xv, Wq, Wk, Wv, Wp):
    bf = ml_dtypes.bfloat16
    in_maps = []
    xT = {}
    for b in range(B):
        xT[b] = tuple(
            np.ascontiguousarray(a[b].T).astype(bf) for a in (xq, xk, xv)
        )
    for c in range(NCORES):
        b, g = c // 4, c % 4
        hs = slice(192 * g, 192 * (g + 1))
        wqk = np.concatenate([Wq[hs].T, Wk[hs].T], axis=1)
        in_maps.append(
            {
                "xqT": xT[b][0],
                "xkT": xT[b][1],
                "xvT": xT[b][2],
                "wqkT": np.ascontiguousarray(wqk).astype(bf),
                "wvT": np.ascontiguousarray(Wv[hs].T).astype(bf),
                "wpT": np.ascontiguousarray(Wp[:, hs].T).astype(bf),
            }
        )
    return in_maps


def _ensure_ntff_hook():
    """Register the axon NTFF profiling hook if the stub antenv lacks it."""
    import types

    try:
        from antenv.axon_hooks import get_axon_ntff_profile_hook  # noqa: F401

        return
    except ImportError:
        pass
    try:
        import antenv
        from trn_agent_boot.trn_boot import _ntff_profile_via_ctypes

        so_path = "/opt/axon/libaxon_pjrt.so"
        hook = _ntff_profile_via_ctypes(so_path) if os.path.exists(so_path) else None
        mod = types.ModuleType("antenv.axon_hooks")
        _state = {"h": hook}
        mod.get_axon_ntff_profile_hook = lambda: _state["h"]
        mod.set_axon_ntff_profile_hook = lambda h: _state.__setitem__("h", h)
        sys.modules["antenv.axon_hooks"] = mod
        antenv.axon_hooks = mod
    except Exception:
        pass


def kernel(xq, xk, xv, Wq, Wk, Wv, Wp, bp):
    global LAST_RESULT
    from concourse.bass_utils import run_bass_kernel_spmd

    key = "nc"
    if key not in _BUILT:
        _BUILT[key] = build_bass()
    nc = _BUILT[key]

    xq, xk, xv = (np.asarray(a, np.float32) for a in (xq, xk, xv))
    Wq, Wk, Wv, Wp = (np.asarray(a, np.float32) for a in (Wq, Wk, Wv, Wp))
    bp = np.asarray(bp, np.float32)

    in_maps = _shard_inputs(xq, xk, xv, Wq, Wk, Wv, Wp)
    trace = bool(os.environ.get("BASS_KERNEL_TRACE"))
    if trace:
        _ensure_ntff_hook()
    res = run_bass_kernel_spmd(
        nc, in_maps, core_ids=list(range(NCORES)), trace=trace
    )
    LAST_RESULT = res
    parts = [res.results[i]["out"].astype(np.float32) for i in range(NCORES)]
    out = np.stack(
        [
            parts[0] + parts[1] + parts[2] + parts[3],
            parts[4] + parts[5] + parts[6] + parts[7],
        ]
    )
    return (out + bp[None, None, :]).astype(np.float32)
